# revision 2
# baseline (speedup 1.0000x reference)
"""Causal attention kernel for Trainium2 (Bass/Tile), 8-core SPMD.

Problem: B=16, S=2048, D=128 fp32 causal attention
    scores = Q @ K^T            (per batch)
    scores -= INF * triu(k=1)   (before scaling, as in reference)
    attn = softmax(scores / sqrt(D))
    out = attn @ V

Sharding: batch dim across 8 cores, 2 batches per core, no communication.

Per-core dataflow (per batch, per 512-wide q-block, per 128-wide k-chunk):
    S^T[k, q] = (K^T chunk).T @ Q^T slice      (contract d on partitions)
    diag chunks: += triangular -1e9 mask (DVE)
    P^T = exp(S^T * 1/sqrt(D))                 (ACT, PSUM -> SBUF fp32r)
    O^T[d, q] += V_chunk.T @ P^T chunk          (PSUM accumulate)
    l[q]     += ones.T @ P^T chunk              (rowsum)
    out[q, d] = transpose(O^T) * (1/l)          (PE transpose + DVE scale)

All matmuls run in fp32r (full-rate fp32 on the PE; producers round).
"""

import os

os.environ.setdefault("MYCRO_LOCAL_CACHE", "1")

import math

import numpy as np

import concourse.bass as bass
import concourse.mybir as mybir
import concourse.tile as tile
from concourse import bacc
from concourse.bass_utils import run_bass_kernel_spmd
from concourse.masks import make_identity

F32 = mybir.dt.float32
F32R = mybir.dt.float32r
EXPF = mybir.ActivationFunctionType.Exp

N_CORES = 8
B = 16
S = 2048
D = 128
BPC = B // N_CORES  # batches per core
SCALE = 1.0 / math.sqrt(float(D))
NEG = -1.0e9
NQB = S // 512  # q blocks per batch
NCH = S // 128  # k chunks per batch

# diag group packing: chunk m has width 512-128m, packed at bank-aligned col
DIAG_COLS = [0, 512, 1024, 1280]
DIAG_EXTENT = 1408


def build():
    nc = bacc.Bacc("TRN2", target_bir_lowering=False, debug=False, num_devices=N_CORES)
    q_d = nc.dram_tensor("q", [BPC, S, D], F32, kind="ExternalInput")
    k_d = nc.dram_tensor("k", [BPC, S, D], F32, kind="ExternalInput")
    v_d = nc.dram_tensor("v", [BPC, S, D], F32, kind="ExternalInput")
    o_d = nc.dram_tensor("o", [BPC, S, D], F32, kind="ExternalOutput")

    with tile.TileContext(nc) as tc:
        with (
            tc.tile_pool(name="const", bufs=1) as constp,
            tc.tile_pool(name="nat", bufs=2) as natp,
            tc.tile_pool(name="tpose", bufs=2) as tposep,
            tc.tile_pool(name="pt", bufs=3) as ptp,
            tc.tile_pool(name="evac", bufs=2) as evacp,
            tc.tile_pool(name="stps", bufs=2, space="PSUM") as stps,
            tc.tile_pool(name="otps", bufs=1, space="PSUM") as otps,
            tc.tile_pool(name="lps", bufs=1, space="PSUM") as lps,
        ):
            # ---- constants ----
            ident32 = constp.tile([128, 128], F32, name="ident32")
            make_identity(nc, ident32[:])
            mask = constp.tile([128, 128], F32, name="mask")
            nc.gpsimd.memset(mask[:], 0.0)
            # keep 0 where q(free) >= k(partition), else NEG
            nc.gpsimd.affine_select(
                out=mask[:],
                in_=mask[:],
                compare_op=mybir.AluOpType.is_ge,
                fill=NEG,
                base=0,
                pattern=[[1, 128]],
                channel_multiplier=-1,
            )
            zb = constp.tile([128, 1], F32, name="zb")
            nc.gpsimd.memset(zb[:], 0.0)
            ones_f = constp.tile([128, 1], F32, name="ones_f")
            nc.gpsimd.memset(ones_f[:], 1.0)
            ones_r = constp.tile([128, 1], F32R, name="ones_r")
            nc.vector.tensor_copy(ones_r[:], ones_f[:])

            for b in range(BPC):
                # ---- load natural-layout tiles ----
                # nat[:, j*128 + d] = X[b, j*128 + p, d]
                q_nat = natp.tile([128, S], F32, name="q_nat")
                k_nat = natp.tile([128, S], F32, name="k_nat")
                v_nat = natp.tile([128, S], F32, name="v_nat")
                nc.sync.dma_start(
                    q_nat[:].rearrange("p (j d) -> p j d", d=128),
                    q_d[b].rearrange("(j p) d -> p j d", p=128),
                )
                nc.sync.dma_start(
                    k_nat[:].rearrange("p (j d) -> p j d", d=128),
                    k_d[b].rearrange("(j p) d -> p j d", p=128),
                )
                nc.sync.dma_start(
                    v_nat[:].rearrange("p (j d) -> p j d", d=128),
                    v_d[b].rearrange("(j p) d -> p j d", p=128),
                )

                # ---- build Q^T, K^T (fp32r) via PE transposes ----
                qt = tposep.tile([128, S], F32R, name="qt")
                kt = tposep.tile([128, S], F32R, name="kt")
                for (nat, tt) in ((q_nat, qt), (k_nat, kt)):
                    for j4 in range(NCH // 4):
                        tp = stps.tile([128, 512], F32, name="tp", tag="stps")
                        for m in range(4):
                            j = j4 * 4 + m
                            nc.tensor.transpose(
                                tp[:, m * 128 : (m + 1) * 128],
                                nat[:, j * 128 : (j + 1) * 128],
                                ident32[:],
                            )
                        nc.vector.tensor_copy(
                            tt[:, j4 * 512 : (j4 + 1) * 512], tp[:]
                        )

                # ---- round V to fp32r ----
                vr = tposep.tile([128, S], F32R, name="vr")
                nc.vector.tensor_copy(vr[:], v_nat[:])

                # ---- q blocks ----
                for qb in range(NQB):
                    n_full = 4 * qb
                    n_ch = n_full + 4
                    q0 = qb * 512

                    # chunk list: (j, qoff, width, col_in_tile) grouped by tile
                    groups = []
                    jf = 0
                    while jf < n_full:
                        g = min(3, n_full - jf)
                        groups.append(
                            (
                                [(jf + c, 0, 512, c * 512) for c in range(g)],
                                g * 512,
                                False,
                            )
                        )
                        jf += g
                    groups.append(
                        (
                            [
                                (n_full + m, 128 * m, 512 - 128 * m, DIAG_COLS[m])
                                for m in range(4)
                            ],
                            DIAG_EXTENT,
                            True,
                        )
                    )

                    ot = otps.tile([128, 512], F32, name="ot")
                    lp = lps.tile([128, 512], F32, name="lp")

                    for chunks, extent, is_diag in groups:
                        st = stps.tile([128, 1536], F32, name="st", tag="stps")
                        for (j, qoff, width, col) in chunks:
                            nc.tensor.matmul(
                                st[:, col : col + width],
                                kt[:, j * 128 : (j + 1) * 128],
                                qt[:, q0 + qoff : q0 + qoff + width],
                                start=True,
                                stop=True,
                            )
                        if is_diag:
                            for (j, qoff, width, col) in chunks:
                                nc.vector.tensor_add(
                                    st[:, col : col + 128],
                                    st[:, col : col + 128],
                                    mask[:],
                                )
                        pt = ptp.tile([128, 1536], F32R, name="pt", tag="pt")
                        nc.scalar.activation(
                            pt[:, 0:extent],
                            st[:, 0:extent],
                            EXPF,
                            bias=zb[:],
                            scale=SCALE,
                        )
                        for (j, qoff, width, col) in chunks:
                            nc.tensor.matmul(
                                ot[:, qoff : qoff + width],
                                vr[:, j * 128 : (j + 1) * 128],
                                pt[:, col : col + width],
                                start=(j == 0),
                                stop=(j == n_ch - 1),
                            )
                            nc.tensor.matmul(
                                lp[0:1, qoff : qoff + width],
                                ones_r[:],
                                pt[:, col : col + width],
                                start=(j == 0),
                                stop=(j == n_ch - 1),
                            )

                    # ---- evacuate: O = transpose(O^T) / l ----
                    ots = evacp.tile([128, 512], F32, name="ots")
                    nc.vector.tensor_copy(ots[:], ot[:])
                    ls = evacp.tile([1, 512], F32, name="ls")
                    nc.vector.tensor_copy(ls[:], lp[0:1, :])

                    # transpose l via K=1 matmuls (fp32): lt[:, s] = l[s*128:+128]
                    lt = stps.tile([128, 4], F32, name="lt", tag="stps")
                    for s4 in range(4):
                        nc.tensor.matmul(
                            lt[:, s4 : s4 + 1],
                            ls[0:1, s4 * 128 : (s4 + 1) * 128],
                            ones_f[0:1, :],
                            start=True,
                            stop=True,
                        )
                    recip = evacp.tile([128, 4], F32, name="recip")
                    nc.vector.reciprocal(recip[:], lt[:])

                    trp = stps.tile([128, 512], F32, name="trp", tag="stps")
                    for s4 in range(4):
                        nc.tensor.transpose(
                            trp[:, s4 * 128 : (s4 + 1) * 128],
                            ots[:, s4 * 128 : (s4 + 1) * 128],
                            ident32[:],
                        )
                    outsb = evacp.tile([128, 512], F32, name="outsb")
                    for s4 in range(4):
                        nc.vector.tensor_scalar_mul(
                            outsb[:, s4 * 128 : (s4 + 1) * 128],
                            trp[:, s4 * 128 : (s4 + 1) * 128],
                            recip[:, s4 : s4 + 1],
                        )
                    nc.sync.dma_start(
                        o_d[b, q0 : q0 + 512, :].rearrange("(a p) d -> p a d", p=128),
                        outsb[:].rearrange("p (a d) -> p a d", d=128),
                    )
    nc.compile()
    return nc


_NC_CACHE = None


def _get_nc():
    global _NC_CACHE
    if _NC_CACHE is None:
        _NC_CACHE = build()
    return _NC_CACHE


def kernel(query, key, value, _trace=False):
    nc = _get_nc()
    in_maps = []
    for c in range(N_CORES):
        sl = slice(c * BPC, (c + 1) * BPC)
        in_maps.append(
            {
                "q": np.ascontiguousarray(query[sl], dtype=np.float32),
                "k": np.ascontiguousarray(key[sl], dtype=np.float32),
                "v": np.ascontiguousarray(value[sl], dtype=np.float32),
            }
        )
    res = run_bass_kernel_spmd(
        nc, in_maps, core_ids=list(range(N_CORES)), trace=_trace
    )
    out = np.concatenate([res.results[c]["o"] for c in range(N_CORES)], axis=0)
    if _trace:
        return out, res
    return out


# revision 5
# speedup vs baseline: 1.0974x; 1.0974x over previous
"""Causal attention kernel for Trainium2 (Bass/Tile), 8-core SPMD.

Problem: B=16, S=2048, D=128 fp32 causal attention
    scores = Q @ K^T            (per batch)
    scores -= INF * triu(k=1)   (before scaling, as in reference)
    attn = softmax(scores / sqrt(D))
    out = attn @ V

Sharding: batch dim across 8 cores, 2 batches per core, no communication.

Per-core dataflow (per batch, per 512-wide q-block, per 128-wide k-chunk):
    S^T[k, q] = (K^T chunk).T @ Q^T slice      (contract d on partitions)
    diag chunks: += triangular -1e9 mask (DVE)
    P^T = exp(S^T * 1/sqrt(D))                 (ACT, PSUM -> SBUF fp32r)
    O^T[d, q] += V_chunk.T @ P^T chunk          (PSUM accumulate)
    l[q]     += ones.T @ P^T chunk              (rowsum)
    out[q, d] = transpose(O^T) * (1/l)          (PE transpose + DVE scale)

All matmuls run in fp32r (full-rate fp32 on the PE; producers round).
"""

import os

os.environ.setdefault("MYCRO_LOCAL_CACHE", "1")

import math

import numpy as np

import concourse.bass as bass
import concourse.mybir as mybir
import concourse.tile as tile
from concourse import bacc
from concourse.bass_utils import run_bass_kernel_spmd
from concourse.masks import make_identity

F32 = mybir.dt.float32
F32R = mybir.dt.float32r
EXPF = mybir.ActivationFunctionType.Exp

N_CORES = 8
B = 16
S = 2048
D = 128
BPC = B // N_CORES  # batches per core
SCALE = 1.0 / math.sqrt(float(D))
NEG = -1.0e9
NQB = S // 512  # q blocks per batch
NCH = S // 128  # k chunks per batch

# diag group packing: chunk m has width 512-128m, packed at bank-aligned col
DIAG_COLS = [0, 512, 1024, 1280]
DIAG_EXTENT = 1408


def build():
    nc = bacc.Bacc("TRN2", target_bir_lowering=False, debug=False, num_devices=N_CORES)
    q_d = nc.dram_tensor("q", [BPC, S, D], F32, kind="ExternalInput")
    k_d = nc.dram_tensor("k", [BPC, S, D], F32, kind="ExternalInput")
    v_d = nc.dram_tensor("v", [BPC, S, D], F32, kind="ExternalInput")
    o_d = nc.dram_tensor("o", [BPC, S, D], F32, kind="ExternalOutput")

    with tile.TileContext(nc) as tc:
        with (
            tc.tile_pool(name="const", bufs=1) as constp,
            tc.tile_pool(name="nat", bufs=2) as natp,
            tc.tile_pool(name="tpose", bufs=2) as tposep,
            tc.tile_pool(name="pt", bufs=3) as ptp,
            tc.tile_pool(name="evac", bufs=2) as evacp,
            tc.tile_pool(name="stps", bufs=2, space="PSUM") as stps,
            tc.tile_pool(name="otps", bufs=2, space="PSUM") as otps,
            tc.tile_pool(name="lps", bufs=1, space="PSUM") as lps,
            tc.tile_pool(name="trps", bufs=1, space="PSUM") as trps,
        ):
            # ---- constants ----
            ident32 = constp.tile([128, 128], F32, name="ident32")
            make_identity(nc, ident32[:])
            mask = constp.tile([128, 128], F32, name="mask")
            nc.gpsimd.memset(mask[:], 0.0)
            # keep 0 where q(free) >= k(partition), else NEG
            nc.gpsimd.affine_select(
                out=mask[:],
                in_=mask[:],
                compare_op=mybir.AluOpType.is_ge,
                fill=NEG,
                base=0,
                pattern=[[1, 128]],
                channel_multiplier=-1,
            )
            zb = constp.tile([128, 1], F32, name="zb")
            nc.gpsimd.memset(zb[:], 0.0)
            ones_f = constp.tile([128, 1], F32, name="ones_f")
            nc.gpsimd.memset(ones_f[:], 1.0)
            ones_r = constp.tile([128, 1], F32R, name="ones_r")
            nc.vector.tensor_copy(ones_r[:], ones_f[:])
            onesrow_f = constp.tile([1, 128], F32, name="onesrow_f")
            nc.gpsimd.memset(onesrow_f[:], 1.0)
            onesrow_r = constp.tile([1, 128], F32R, name="onesrow_r")
            nc.vector.tensor_copy(onesrow_r[:], onesrow_f[:])
            identR = constp.tile([128, 128], F32R, name="identR")
            nc.vector.tensor_copy(identR[:], ident32[:])

            # HAM warmup: dense PE activity while the first DMAs land
            warm_ps = stps.tile([128, 128], F32, name="warm_ps", tag="stps")
            for _ in range(30):
                nc.tensor.matmul(
                    warm_ps[:], identR[:], identR[:], start=True, stop=True
                )

            for b in range(BPC):
                # ---- load natural-layout tiles ----
                # nat[:, j*128 + d] = X[b, j*128 + p, d]
                q_nat = natp.tile([128, S], F32, name="q_nat")
                k_nat = natp.tile([128, S], F32, name="k_nat")
                v_nat = natp.tile([128, S], F32, name="v_nat")
                nc.sync.dma_start(
                    q_nat[:].rearrange("p (j d) -> p j d", d=128),
                    q_d[b].rearrange("(j p) d -> p j d", p=128),
                )
                nc.sync.dma_start(
                    k_nat[:].rearrange("p (j d) -> p j d", d=128),
                    k_d[b].rearrange("(j p) d -> p j d", p=128),
                )
                nc.sync.dma_start(
                    v_nat[:].rearrange("p (j d) -> p j d", d=128),
                    v_d[b].rearrange("(j p) d -> p j d", p=128),
                )

                # ---- build Q^T, K^T (fp32r) via PE transposes ----
                qt = tposep.tile([128, S], F32R, name="qt")
                kt = tposep.tile([128, S], F32R, name="kt")
                for (nat, tt) in ((q_nat, qt), (k_nat, kt)):
                    for j4 in range(NCH // 4):
                        tp = stps.tile([128, 512], F32, name="tp", tag="stps")
                        for m in range(4):
                            j = j4 * 4 + m
                            nc.tensor.transpose(
                                tp[:, m * 128 : (m + 1) * 128],
                                nat[:, j * 128 : (j + 1) * 128],
                                ident32[:],
                            )
                        nc.vector.tensor_copy(
                            tt[:, j4 * 512 : (j4 + 1) * 512], tp[:]
                        )

                # ---- round V to fp32r ----
                vr = tposep.tile([128, S], F32R, name="vr")
                nc.vector.tensor_copy(vr[:], v_nat[:])

                # ---- q blocks ----
                for qb in range(NQB):
                    n_full = 4 * qb
                    n_ch = n_full + 4
                    q0 = qb * 512

                    # chunk list: (j, qoff, width, col_in_tile) grouped by tile
                    groups = []
                    jf = 0
                    while jf < n_full:
                        g = min(2, n_full - jf)
                        groups.append(
                            (
                                [(jf + c, 0, 512, c * 512) for c in range(g)],
                                g * 512,
                                False,
                            )
                        )
                        jf += g
                    # diag chunks m=0..3, widths 512/384/256/128, split 2+2
                    groups.append(
                        (
                            [
                                (n_full + 0, 0, 512, 0),
                                (n_full + 1, 128, 384, 512),
                            ],
                            896,
                            True,
                        )
                    )
                    groups.append(
                        (
                            [
                                (n_full + 2, 256, 256, 0),
                                (n_full + 3, 384, 128, 256),
                            ],
                            384,
                            True,
                        )
                    )

                    ot = otps.tile([128, 512], F32, name="ot")
                    lp = lps.tile([128, 512], F32, name="lp", tag="lp")

                    for chunks, extent, is_diag in groups:
                        st = stps.tile([128, 1024], F32, name="st", tag="stps")
                        for (j, qoff, width, col) in chunks:
                            nc.tensor.matmul(
                                st[:, col : col + width],
                                kt[:, j * 128 : (j + 1) * 128],
                                qt[:, q0 + qoff : q0 + qoff + width],
                                start=True,
                                stop=True,
                            )
                        if is_diag:
                            for (j, qoff, width, col) in chunks:
                                nc.vector.tensor_add(
                                    st[:, col : col + 128],
                                    st[:, col : col + 128],
                                    mask[:],
                                )
                        pt = ptp.tile([128, 1024], F32R, name="pt", tag="pt")
                        nc.scalar.activation(
                            pt[:, 0:extent],
                            st[:, 0:extent],
                            EXPF,
                            bias=zb[:],
                            scale=SCALE,
                        )
                        for (j, qoff, width, col) in chunks:
                            nc.tensor.matmul(
                                ot[:, qoff : qoff + width],
                                vr[:, j * 128 : (j + 1) * 128],
                                pt[:, col : col + width],
                                start=(j == 0),
                                stop=(j == n_ch - 1),
                            )
                            nc.tensor.matmul(
                                lp[0:1, qoff : qoff + width],
                                ones_r[:],
                                pt[:, col : col + width],
                                start=(j == 0),
                                stop=(j == n_ch - 1),
                            )

                    # ---- evacuate: O = transpose(O^T * bcast(1/l)) ----
                    recip = evacp.tile([1, 512], F32, name="recip")
                    nc.vector.reciprocal(recip[:], lp[0:1, :])
                    bc_sb = evacp.tile([128, 512], F32, name="bc_sb")
                    nc.gpsimd.partition_broadcast(bc_sb[:], recip[:])
                    ots = evacp.tile([128, 512], F32R, name="ots")
                    with nc.allow_low_precision("f32r is full-width fp32 storage"):
                        nc.vector.tensor_mul(ots[:], ot[:], bc_sb[:])

                    trp = trps.tile([128, 512], F32R, name="trp")
                    for s4 in range(4):
                        nc.tensor.transpose(
                            trp[:, s4 * 128 : (s4 + 1) * 128],
                            ots[:, s4 * 128 : (s4 + 1) * 128],
                            identR[:],
                        )
                    outsb = evacp.tile([128, 512], F32, name="outsb")
                    nc.vector.tensor_copy(outsb[:], trp[:].bitcast(F32))
                    nc.sync.dma_start(
                        o_d[b, q0 : q0 + 512, :].rearrange("(a p) d -> p a d", p=128),
                        outsb[:].rearrange("p (a d) -> p a d", d=128),
                    )
    nc.compile()
    return nc


_NC_CACHE = None


def _get_nc():
    global _NC_CACHE
    if _NC_CACHE is None:
        _NC_CACHE = build()
    return _NC_CACHE


def kernel(query, key, value, _trace=False):
    nc = _get_nc()
    in_maps = []
    for c in range(N_CORES):
        sl = slice(c * BPC, (c + 1) * BPC)
        in_maps.append(
            {
                "q": np.ascontiguousarray(query[sl], dtype=np.float32),
                "k": np.ascontiguousarray(key[sl], dtype=np.float32),
                "v": np.ascontiguousarray(value[sl], dtype=np.float32),
            }
        )
    res = run_bass_kernel_spmd(
        nc, in_maps, core_ids=list(range(N_CORES)), trace=_trace
    )
    out = np.concatenate([res.results[c]["o"] for c in range(N_CORES)], axis=0)
    if _trace:
        return out, res
    return out


# revision 6
# speedup vs baseline: 1.1199x; 1.0205x over previous
"""Causal attention kernel for Trainium2 (Bass/Tile), 8-core SPMD.

Problem: B=16, S=2048, D=128 fp32 causal attention
    scores = Q @ K^T            (per batch)
    scores -= INF * triu(k=1)   (before scaling, as in reference)
    attn = softmax(scores / sqrt(D))
    out = attn @ V

Sharding: batch dim across 8 cores, 2 batches per core, no communication.

Per-core dataflow (per batch, per 512-wide q-block, per 128-wide k-chunk):
    S^T[k, q] = (K^T chunk).T @ Q^T slice      (contract d on partitions)
    diag chunks: += triangular -1e9 mask (DVE)
    P^T = exp(S^T * 1/sqrt(D))                 (ACT, PSUM -> SBUF fp32r)
    O^T[d, q] += V_chunk.T @ P^T chunk          (PSUM accumulate)
    l[q]     += ones.T @ P^T chunk              (rowsum)
    out[q, d] = transpose(O^T) * (1/l)          (PE transpose + DVE scale)

All matmuls run in fp32r (full-rate fp32 on the PE; producers round).
"""

import os

os.environ.setdefault("MYCRO_LOCAL_CACHE", "1")

import math

import numpy as np

import concourse.bass as bass
import concourse.mybir as mybir
import concourse.tile as tile
from concourse import bacc
from concourse.bass_utils import run_bass_kernel_spmd
from concourse.masks import make_identity

F32 = mybir.dt.float32
F32R = mybir.dt.float32r
EXPF = mybir.ActivationFunctionType.Exp

N_CORES = 8
B = 16
S = 2048
D = 128
BPC = B // N_CORES  # batches per core
SCALE = 1.0 / math.sqrt(float(D))
NEG = -1.0e9
NQB = S // 512  # q blocks per batch
NCH = S // 128  # k chunks per batch

# diag group packing: chunk m has width 512-128m, packed at bank-aligned col
DIAG_COLS = [0, 512, 1024, 1280]
DIAG_EXTENT = 1408


def build():
    nc = bacc.Bacc("TRN2", target_bir_lowering=False, debug=False, num_devices=N_CORES)
    q_d = nc.dram_tensor("q", [BPC, S, D], F32, kind="ExternalInput")
    k_d = nc.dram_tensor("k", [BPC, S, D], F32, kind="ExternalInput")
    v_d = nc.dram_tensor("v", [BPC, S, D], F32, kind="ExternalInput")
    o_d = nc.dram_tensor("o", [BPC, S, D], F32, kind="ExternalOutput")

    with tile.TileContext(nc) as tc:
        with (
            tc.tile_pool(name="const", bufs=1) as constp,
            tc.tile_pool(name="nat", bufs=2) as natp,
            tc.tile_pool(name="tpose", bufs=2) as tposep,
            tc.tile_pool(name="pt", bufs=3) as ptp,
            tc.tile_pool(name="evac", bufs=2) as evacp,
            tc.tile_pool(name="stps", bufs=2, space="PSUM") as stps,
            tc.tile_pool(name="otps", bufs=2, space="PSUM") as otps,
            tc.tile_pool(name="lps", bufs=1, space="PSUM") as lps,
            tc.tile_pool(name="trps", bufs=1, space="PSUM") as trps,
        ):
            # ---- constants ----
            ident32 = constp.tile([128, 128], F32, name="ident32")
            make_identity(nc, ident32[:])
            mask = constp.tile([128, 128], F32, name="mask")
            nc.gpsimd.memset(mask[:], 0.0)
            # keep 0 where q(free) >= k(partition), else NEG
            nc.gpsimd.affine_select(
                out=mask[:],
                in_=mask[:],
                compare_op=mybir.AluOpType.is_ge,
                fill=NEG,
                base=0,
                pattern=[[1, 128]],
                channel_multiplier=-1,
            )
            zb = constp.tile([128, 1], F32, name="zb")
            nc.gpsimd.memset(zb[:], 0.0)
            ones_f = constp.tile([128, 128], F32, name="ones_f")
            nc.gpsimd.memset(ones_f[:], 1.0)
            ones_r = constp.tile([128, 128], F32R, name="ones_r")
            nc.vector.tensor_copy(ones_r[:], ones_f[:])
            identR = constp.tile([128, 128], F32R, name="identR")
            nc.vector.tensor_copy(identR[:], ident32[:])

            # HAM warmup: dense PE activity while the first DMAs land
            warm_ps = stps.tile([128, 128], F32, name="warm_ps", tag="stps")
            for _ in range(30):
                nc.tensor.matmul(
                    warm_ps[:], identR[:], identR[:], start=True, stop=True
                )

            for b in range(BPC):
                # ---- load natural-layout tiles ----
                # nat[:, j*128 + d] = X[b, j*128 + p, d]
                q_nat = natp.tile([128, S], F32, name="q_nat")
                k_nat = natp.tile([128, S], F32, name="k_nat")
                v_nat = natp.tile([128, S], F32, name="v_nat")
                nc.sync.dma_start(
                    q_nat[:].rearrange("p (j d) -> p j d", d=128),
                    q_d[b].rearrange("(j p) d -> p j d", p=128),
                )
                nc.sync.dma_start(
                    k_nat[:].rearrange("p (j d) -> p j d", d=128),
                    k_d[b].rearrange("(j p) d -> p j d", p=128),
                )
                nc.sync.dma_start(
                    v_nat[:].rearrange("p (j d) -> p j d", d=128),
                    v_d[b].rearrange("(j p) d -> p j d", p=128),
                )

                # ---- build Q^T, K^T (fp32r) via PE transposes ----
                qt = tposep.tile([128, S], F32R, name="qt")
                kt = tposep.tile([128, S], F32R, name="kt")
                for (nat, tt) in ((q_nat, qt), (k_nat, kt)):
                    for j4 in range(NCH // 4):
                        tp = stps.tile([128, 512], F32, name="tp", tag="stps")
                        for m in range(4):
                            j = j4 * 4 + m
                            nc.tensor.transpose(
                                tp[:, m * 128 : (m + 1) * 128],
                                nat[:, j * 128 : (j + 1) * 128],
                                ident32[:],
                            )
                        nc.vector.tensor_copy(
                            tt[:, j4 * 512 : (j4 + 1) * 512], tp[:]
                        )

                # ---- round V to fp32r ----
                vr = tposep.tile([128, S], F32R, name="vr")
                nc.vector.tensor_copy(vr[:], v_nat[:])

                # ---- q blocks ----
                for qb in range(NQB):
                    n_full = 4 * qb
                    n_ch = n_full + 4
                    q0 = qb * 512

                    # chunk list: (j, qoff, width, col_in_tile) grouped by tile
                    groups = []
                    jf = 0
                    while jf < n_full:
                        g = min(2, n_full - jf)
                        groups.append(
                            (
                                [(jf + c, 0, 512, c * 512) for c in range(g)],
                                g * 512,
                                False,
                            )
                        )
                        jf += g
                    # diag chunks m=0..3, widths 512/384/256/128, split 2+2
                    groups.append(
                        (
                            [
                                (n_full + 0, 0, 512, 0),
                                (n_full + 1, 128, 384, 512),
                            ],
                            896,
                            True,
                        )
                    )
                    groups.append(
                        (
                            [
                                (n_full + 2, 256, 256, 0),
                                (n_full + 3, 384, 128, 256),
                            ],
                            384,
                            True,
                        )
                    )

                    ot = otps.tile([128, 512], F32, name="ot")
                    lp = lps.tile([128, 512], F32, name="lp", tag="lp")

                    for chunks, extent, is_diag in groups:
                        st = stps.tile([128, 1024], F32, name="st", tag="stps")
                        for (j, qoff, width, col) in chunks:
                            nc.tensor.matmul(
                                st[:, col : col + width],
                                kt[:, j * 128 : (j + 1) * 128],
                                qt[:, q0 + qoff : q0 + qoff + width],
                                start=True,
                                stop=True,
                            )
                        if is_diag:
                            for (j, qoff, width, col) in chunks:
                                nc.vector.tensor_add(
                                    st[:, col : col + 128],
                                    st[:, col : col + 128],
                                    mask[:],
                                )
                        pt = ptp.tile([128, 1024], F32R, name="pt", tag="pt")
                        nc.scalar.activation(
                            pt[:, 0:extent],
                            st[:, 0:extent],
                            EXPF,
                            bias=zb[:],
                            scale=SCALE,
                        )
                        for (j, qoff, width, col) in chunks:
                            nc.tensor.matmul(
                                ot[:, qoff : qoff + width],
                                vr[:, j * 128 : (j + 1) * 128],
                                pt[:, col : col + width],
                                start=(j == 0),
                                stop=(j == n_ch - 1),
                            )
                            nc.tensor.matmul(
                                lp[:, qoff : qoff + width],
                                ones_r[:],
                                pt[:, col : col + width],
                                start=(j == 0),
                                stop=(j == n_ch - 1),
                            )

                    # ---- evacuate: O = transpose(O^T * (1/l)) ----
                    # lp rows are all equal (all-ones stationary), so recip
                    # and the scale run full-width on DVE.
                    recip = evacp.tile([128, 512], F32, name="recip")
                    nc.vector.reciprocal(recip[:], lp[:])
                    ots = evacp.tile([128, 512], F32R, name="ots")
                    with nc.allow_low_precision("f32r is full-width fp32 storage"):
                        nc.vector.tensor_mul(ots[:], ot[:], recip[:])

                    trp = trps.tile([128, 512], F32R, name="trp")
                    for s4 in range(4):
                        nc.tensor.transpose(
                            trp[:, s4 * 128 : (s4 + 1) * 128],
                            ots[:, s4 * 128 : (s4 + 1) * 128],
                            identR[:],
                        )
                    outsb = evacp.tile([128, 512], F32, name="outsb")
                    nc.vector.tensor_copy(outsb[:], trp[:].bitcast(F32))
                    nc.sync.dma_start(
                        o_d[b, q0 : q0 + 512, :].rearrange("(a p) d -> p a d", p=128),
                        outsb[:].rearrange("p (a d) -> p a d", d=128),
                    )
    nc.compile()
    return nc


_NC_CACHE = None


def _get_nc():
    global _NC_CACHE
    if _NC_CACHE is None:
        _NC_CACHE = build()
    return _NC_CACHE


def kernel(query, key, value, _trace=False):
    nc = _get_nc()
    in_maps = []
    for c in range(N_CORES):
        sl = slice(c * BPC, (c + 1) * BPC)
        in_maps.append(
            {
                "q": np.ascontiguousarray(query[sl], dtype=np.float32),
                "k": np.ascontiguousarray(key[sl], dtype=np.float32),
                "v": np.ascontiguousarray(value[sl], dtype=np.float32),
            }
        )
    res = run_bass_kernel_spmd(
        nc, in_maps, core_ids=list(range(N_CORES)), trace=_trace
    )
    out = np.concatenate([res.results[c]["o"] for c in range(N_CORES)], axis=0)
    if _trace:
        return out, res
    return out


# revision 7
# speedup vs baseline: 1.2201x; 1.0895x over previous
"""Causal attention kernel for Trainium2 (Bass/Tile), 8-core SPMD.

Problem: B=16, S=2048, D=128 fp32 causal attention
    scores = Q @ K^T            (per batch)
    scores -= INF * triu(k=1)   (before scaling, as in reference)
    attn = softmax(scores / sqrt(D))
    out = attn @ V

Sharding: batch dim across 8 cores, 2 batches per core, no communication.

Per-core dataflow (per batch, per 512-wide q-block, per 128-wide k-chunk):
    S^T[k, q] = (K^T chunk).T @ Q^T slice      (contract d on partitions)
    diag chunks: += triangular -1e9 mask (DVE)
    P^T = exp(S^T * 1/sqrt(D))                 (ACT, PSUM -> SBUF fp32r)
    O^T[d, q] += V_chunk.T @ P^T chunk          (PSUM accumulate)
    l[q]      += allones.T @ P^T chunk          (rowsum, broadcast on all rows)
    out[q, d] = transpose(O^T * (1/l))          (DVE scale + PE transpose)

All matmuls run in fp32r (full-rate fp32 on the PE; producers round).
Evacuation of q-block N is deferred past q-block N+1's first group so the
in-order PE queue never stalls on the DVE normalize chain.
"""

import os

os.environ.setdefault("MYCRO_LOCAL_CACHE", "1")

import math

import numpy as np

import concourse.bass as bass
import concourse.mybir as mybir
import concourse.tile as tile
from concourse import bacc
from concourse.bass_utils import run_bass_kernel_spmd
from concourse.masks import make_identity

F32 = mybir.dt.float32
F32R = mybir.dt.float32r
EXPF = mybir.ActivationFunctionType.Exp

N_CORES = 8
B = 16
S = 2048
D = 128
BPC = B // N_CORES  # batches per core
SCALE = 1.0 / math.sqrt(float(D))
NEG = -1.0e9
NQB = S // 512  # q blocks per batch
NCH = S // 128  # k chunks per batch


def build():
    nc = bacc.Bacc("TRN2", target_bir_lowering=False, debug=False, num_devices=N_CORES)
    q_d = nc.dram_tensor("q", [BPC, S, D], F32, kind="ExternalInput")
    k_d = nc.dram_tensor("k", [BPC, S, D], F32, kind="ExternalInput")
    v_d = nc.dram_tensor("v", [BPC, S, D], F32, kind="ExternalInput")
    o_d = nc.dram_tensor("o", [BPC, S, D], F32, kind="ExternalOutput")

    with tile.TileContext(nc) as tc:
        with (
            tc.tile_pool(name="const", bufs=1) as constp,
            tc.tile_pool(name="nat", bufs=2) as natp,
            tc.tile_pool(name="tpose", bufs=2) as tposep,
            tc.tile_pool(name="pt", bufs=3) as ptp,
            tc.tile_pool(name="evac", bufs=2) as evacp,
            tc.tile_pool(name="stps", bufs=2, space="PSUM") as stps,
            tc.tile_pool(name="otps", bufs=2, space="PSUM") as otps,
            tc.tile_pool(name="lps", bufs=2, space="PSUM") as lps,
        ):
            # ---- constants ----
            ident32 = constp.tile([128, 128], F32, name="ident32")
            make_identity(nc, ident32[:])
            mask = constp.tile([128, 128], F32, name="mask")
            nc.gpsimd.memset(mask[:], 0.0)
            # keep 0 where q(free) >= k(partition), else NEG
            nc.gpsimd.affine_select(
                out=mask[:],
                in_=mask[:],
                compare_op=mybir.AluOpType.is_ge,
                fill=NEG,
                base=0,
                pattern=[[1, 128]],
                channel_multiplier=-1,
            )
            zb = constp.tile([128, 1], F32, name="zb")
            nc.gpsimd.memset(zb[:], 0.0)
            ones_f = constp.tile([128, 128], F32, name="ones_f")
            nc.gpsimd.memset(ones_f[:], 1.0)
            ones_r = constp.tile([128, 128], F32R, name="ones_r")
            nc.vector.tensor_copy(ones_r[:], ones_f[:])
            identR = constp.tile([128, 128], F32R, name="identR")
            nc.vector.tensor_copy(identR[:], ident32[:])

            # HAM warmup: dense PE activity while the first DMAs land
            warm_ps = stps.tile([128, 128], F32, name="warm_ps", tag="stps")
            for _ in range(30):
                nc.tensor.matmul(
                    warm_ps[:], identR[:], identR[:], start=True, stop=True
                )

            pending_evac = [None]  # deferred PE/DVE tail of the previous q block

            def flush_evac():
                if pending_evac[0] is not None:
                    pending_evac[0]()
                    pending_evac[0] = None

            for b in range(BPC):
                # ---- load natural-layout tiles ----
                # nat[:, j*128 + d] = X[b, j*128 + p, d]
                q_nat = natp.tile([128, S], F32, name="q_nat")
                k_nat = natp.tile([128, S], F32, name="k_nat")
                v_nat = natp.tile([128, S], F32, name="v_nat")
                nc.sync.dma_start(
                    q_nat[:].rearrange("p (j d) -> p j d", d=128),
                    q_d[b].rearrange("(j p) d -> p j d", p=128),
                )
                nc.sync.dma_start(
                    k_nat[:].rearrange("p (j d) -> p j d", d=128),
                    k_d[b].rearrange("(j p) d -> p j d", p=128),
                )
                nc.sync.dma_start(
                    v_nat[:].rearrange("p (j d) -> p j d", d=128),
                    v_d[b].rearrange("(j p) d -> p j d", p=128),
                )

                # ---- build Q^T, K^T (fp32r) via PE transposes ----
                qt = tposep.tile([128, S], F32R, name="qt")
                kt = tposep.tile([128, S], F32R, name="kt")
                first_tgroup = True
                for (nat, tt) in ((q_nat, qt), (k_nat, kt)):
                    for j4 in range(NCH // 4):
                        tp = stps.tile([128, 512], F32, name="tp", tag="stps")
                        for m in range(4):
                            j = j4 * 4 + m
                            nc.tensor.transpose(
                                tp[:, m * 128 : (m + 1) * 128],
                                nat[:, j * 128 : (j + 1) * 128],
                                ident32[:],
                            )
                        nc.vector.tensor_copy(
                            tt[:, j4 * 512 : (j4 + 1) * 512], tp[:]
                        )
                        if first_tgroup:
                            # previous batch's last q-block tail overlaps
                            # this batch's transpose work
                            first_tgroup = False
                            flush_evac()

                # ---- round V to fp32r ----
                vr = tposep.tile([128, S], F32R, name="vr")
                nc.vector.tensor_copy(vr[:], v_nat[:])

                # ---- q blocks ----
                for qb in range(NQB):
                    n_full = 4 * qb
                    n_ch = n_full + 4
                    q0 = qb * 512

                    # (chunks, extent, is_diag); st tiles are [128, 1024]
                    groups = []
                    jf = 0
                    while jf < n_full:
                        g = min(2, n_full - jf)
                        groups.append(
                            (
                                [(jf + c, 0, 512, c * 512) for c in range(g)],
                                g * 512,
                                False,
                            )
                        )
                        jf += g
                    groups.append(
                        ([(n_full, 0, 512, 0), (n_full + 1, 128, 384, 512)], 896, True)
                    )
                    groups.append(
                        (
                            [(n_full + 2, 256, 256, 0), (n_full + 3, 384, 128, 256)],
                            384,
                            True,
                        )
                    )

                    ot = otps.tile([128, 512], F32, name="ot")
                    lp = lps.tile([128, 512], F32, name="lp", tag="lp")

                    for gi, (chunks, extent, is_diag) in enumerate(groups):
                        st = stps.tile([128, 1024], F32, name="st", tag="stps")
                        for (j, qoff, width, col) in chunks:
                            nc.tensor.matmul(
                                st[:, col : col + width],
                                kt[:, j * 128 : (j + 1) * 128],
                                qt[:, q0 + qoff : q0 + qoff + width],
                                start=True,
                                stop=True,
                            )
                        if is_diag:
                            for (j, qoff, width, col) in chunks:
                                nc.vector.tensor_add(
                                    st[:, col : col + 128],
                                    st[:, col : col + 128],
                                    mask[:],
                                )
                        pt = ptp.tile([128, 1024], F32R, name="pt", tag="pt")
                        nc.scalar.activation(
                            pt[:, 0:extent],
                            st[:, 0:extent],
                            EXPF,
                            bias=zb[:],
                            scale=SCALE,
                        )
                        for (j, qoff, width, col) in chunks:
                            nc.tensor.matmul(
                                ot[:, qoff : qoff + width],
                                vr[:, j * 128 : (j + 1) * 128],
                                pt[:, col : col + width],
                                start=(j == 0),
                                stop=(j == n_ch - 1),
                            )
                            nc.tensor.matmul(
                                lp[:, qoff : qoff + width],
                                ones_r[:],
                                pt[:, col : col + width],
                                start=(j == 0),
                                stop=(j == n_ch - 1),
                            )
                        if gi == 0:
                            # PE queue now has fresh work; release the
                            # previous q block's deferred tail behind it.
                            flush_evac()

                    # ---- evacuation stage 1 (DVE): O^T * (1/l) ----
                    # lp rows are all equal (all-ones stationary), so recip
                    # and the scale run full-width.
                    recip = evacp.tile([128, 512], F32, name="recip")
                    nc.vector.reciprocal(recip[:], lp[:])
                    ots = evacp.tile([128, 512], F32R, name="ots")
                    with nc.allow_low_precision("f32r is full-width fp32 storage"):
                        nc.vector.tensor_mul(ots[:], ot[:], recip[:])

                    # ---- evacuation stage 2 (deferred): transpose + store ----
                    def evac(b=b, q0=q0, ots=ots):
                        trp = lps.tile([128, 512], F32R, name="trp", tag="lp")
                        for s4 in range(4):
                            nc.tensor.transpose(
                                trp[:, s4 * 128 : (s4 + 1) * 128],
                                ots[:, s4 * 128 : (s4 + 1) * 128],
                                identR[:],
                            )
                        outsb = evacp.tile([128, 512], F32, name="outsb")
                        nc.vector.tensor_copy(outsb[:], trp[:].bitcast(F32))
                        nc.sync.dma_start(
                            o_d[b, q0 : q0 + 512, :].rearrange(
                                "(a p) d -> p a d", p=128
                            ),
                            outsb[:].rearrange("p (a d) -> p a d", d=128),
                        )

                    pending_evac[0] = evac

            flush_evac()
    nc.compile()
    return nc


_NC_CACHE = None


def _get_nc():
    global _NC_CACHE
    if _NC_CACHE is None:
        _NC_CACHE = build()
    return _NC_CACHE


def kernel(query, key, value, _trace=False):
    nc = _get_nc()
    in_maps = []
    for c in range(N_CORES):
        sl = slice(c * BPC, (c + 1) * BPC)
        in_maps.append(
            {
                "q": np.ascontiguousarray(query[sl], dtype=np.float32),
                "k": np.ascontiguousarray(key[sl], dtype=np.float32),
                "v": np.ascontiguousarray(value[sl], dtype=np.float32),
            }
        )
    res = run_bass_kernel_spmd(
        nc, in_maps, core_ids=list(range(N_CORES)), trace=_trace
    )
    out = np.concatenate([res.results[c]["o"] for c in range(N_CORES)], axis=0)
    if _trace:
        return out, res
    return out


# revision 8
# speedup vs baseline: 1.2667x; 1.0382x over previous
"""Causal attention kernel for Trainium2 (Bass/Tile), 8-core SPMD.

Problem: B=16, S=2048, D=128 fp32 causal attention
    scores = Q @ K^T            (per batch)
    scores -= INF * triu(k=1)   (before scaling, as in reference)
    attn = softmax(scores / sqrt(D))
    out = attn @ V

Sharding: batch dim across 8 cores, 2 batches per core, no communication.

Per-core dataflow (per batch, per 512-wide q-block, per 128-wide k-chunk):
    S^T[k, q] = (K^T chunk).T @ Q^T slice      (contract d on partitions)
    diag chunks: += triangular -1e9 mask (DVE)
    P^T = exp(S^T * 1/sqrt(D))                 (ACT, PSUM -> SBUF fp32r)
    O^T[d, q] += V_chunk.T @ P^T chunk          (PSUM accumulate)
    l[q]      += allones.T @ P^T chunk          (rowsum, broadcast on all rows)
    out[q, d] = transpose(O^T * (1/l))          (DVE scale + PE transpose)

All matmuls run in fp32r (full-rate fp32 on the PE; producers round).
Evacuation of q-block N is deferred past q-block N+1's first group so the
in-order PE queue never stalls on the DVE normalize chain.
"""

import os

os.environ.setdefault("MYCRO_LOCAL_CACHE", "1")

import math

import numpy as np

import concourse.bass as bass
import concourse.mybir as mybir
import concourse.tile as tile
from concourse import bacc
from concourse.bass_utils import run_bass_kernel_spmd
from concourse.masks import make_identity

F32 = mybir.dt.float32
F32R = mybir.dt.float32r
EXPF = mybir.ActivationFunctionType.Exp

N_CORES = 8
B = 16
S = 2048
D = 128
BPC = B // N_CORES  # batches per core
SCALE = 1.0 / math.sqrt(float(D))
NEG = -1.0e9
NQB = S // 512  # q blocks per batch
NCH = S // 128  # k chunks per batch


def build():
    nc = bacc.Bacc("TRN2", target_bir_lowering=False, debug=False, num_devices=N_CORES)
    q_d = nc.dram_tensor("q", [BPC, S, D], F32, kind="ExternalInput")
    k_d = nc.dram_tensor("k", [BPC, S, D], F32, kind="ExternalInput")
    v_d = nc.dram_tensor("v", [BPC, S, D], F32, kind="ExternalInput")
    o_d = nc.dram_tensor("o", [BPC, S, D], F32, kind="ExternalOutput")

    with tile.TileContext(nc) as tc:
        with (
            tc.tile_pool(name="const", bufs=1) as constp,
            tc.tile_pool(name="nat", bufs=2) as natp,
            tc.tile_pool(name="tpose", bufs=2) as tposep,
            tc.tile_pool(name="pt", bufs=3) as ptp,
            tc.tile_pool(name="evac", bufs=2) as evacp,
            tc.tile_pool(name="stps", bufs=2, space="PSUM") as stps,
            tc.tile_pool(name="otps", bufs=2, space="PSUM") as otps,
            tc.tile_pool(name="lps", bufs=2, space="PSUM") as lps,
        ):
            # ---- constants ----
            ident32 = constp.tile([128, 128], F32, name="ident32")
            make_identity(nc, ident32[:])
            mask = constp.tile([128, 128], F32, name="mask")
            nc.gpsimd.memset(mask[:], 0.0)
            # keep 0 where q(free) >= k(partition), else NEG
            nc.gpsimd.affine_select(
                out=mask[:],
                in_=mask[:],
                compare_op=mybir.AluOpType.is_ge,
                fill=NEG,
                base=0,
                pattern=[[1, 128]],
                channel_multiplier=-1,
            )
            zb = constp.tile([128, 1], F32, name="zb")
            nc.gpsimd.memset(zb[:], 0.0)
            ones_f = constp.tile([128, 128], F32, name="ones_f")
            nc.gpsimd.memset(ones_f[:], 1.0)
            ones_r = constp.tile([128, 128], F32R, name="ones_r")
            nc.vector.tensor_copy(ones_r[:], ones_f[:])
            identR = constp.tile([128, 128], F32R, name="identR")
            nc.vector.tensor_copy(identR[:], ident32[:])

            # HAM warmup: dense PE activity while the first DMAs land
            warm_ps = stps.tile([128, 128], F32, name="warm_ps", tag="stps")
            for _ in range(30):
                nc.tensor.matmul(
                    warm_ps[:], identR[:], identR[:], start=True, stop=True
                )

            # software-pipeline state: PV/rowsum of group g is emitted
            # after S+exp of group g+1 (PE never queues behind exp); the
            # transpose/store tail of q-block N is emitted during N+1.
            pending_pv = [None]
            pending_evac = [None]

            def flush_pv():
                if pending_pv[0] is not None:
                    pending_pv[0]()
                    pending_pv[0] = None

            def flush_evac():
                if pending_evac[0] is not None:
                    pending_evac[0]()
                    pending_evac[0] = None

            for b in range(BPC):
                # ---- load natural-layout tiles ----
                # nat[:, j*128 + d] = X[b, j*128 + p, d]
                q_nat = natp.tile([128, S], F32, name="q_nat")
                k_nat = natp.tile([128, S], F32, name="k_nat")
                v_nat = natp.tile([128, S], F32, name="v_nat")
                nc.sync.dma_start(
                    q_nat[:].rearrange("p (j d) -> p j d", d=128),
                    q_d[b].rearrange("(j p) d -> p j d", p=128),
                )
                nc.sync.dma_start(
                    k_nat[:].rearrange("p (j d) -> p j d", d=128),
                    k_d[b].rearrange("(j p) d -> p j d", p=128),
                )
                nc.sync.dma_start(
                    v_nat[:].rearrange("p (j d) -> p j d", d=128),
                    v_d[b].rearrange("(j p) d -> p j d", p=128),
                )

                # ---- build Q^T, K^T (fp32r) via PE transposes ----
                qt = tposep.tile([128, S], F32R, name="qt")
                kt = tposep.tile([128, S], F32R, name="kt")
                tgroup_i = 0
                for (nat, tt) in ((q_nat, qt), (k_nat, kt)):
                    for j4 in range(NCH // 4):
                        tp = stps.tile([128, 512], F32, name="tp", tag="stps")
                        for m in range(4):
                            j = j4 * 4 + m
                            nc.tensor.transpose(
                                tp[:, m * 128 : (m + 1) * 128],
                                nat[:, j * 128 : (j + 1) * 128],
                                ident32[:],
                            )
                        nc.vector.tensor_copy(
                            tt[:, j4 * 512 : (j4 + 1) * 512], tp[:]
                        )
                        # previous batch's tail overlaps the transpose work
                        if tgroup_i == 0:
                            flush_pv()
                        elif tgroup_i == 1:
                            flush_evac()
                        tgroup_i += 1

                # ---- round V to fp32r ----
                vr = tposep.tile([128, S], F32R, name="vr")
                nc.vector.tensor_copy(vr[:], v_nat[:])

                # ---- q blocks ----
                for qb in range(NQB):
                    n_full = 4 * qb
                    n_ch = n_full + 4
                    q0 = qb * 512

                    # (chunks, extent, is_diag); st tiles are [128, 1024]
                    groups = []
                    jf = 0
                    while jf < n_full:
                        g = min(2, n_full - jf)
                        groups.append(
                            (
                                [(jf + c, 0, 512, c * 512) for c in range(g)],
                                g * 512,
                                False,
                            )
                        )
                        jf += g
                    groups.append(
                        ([(n_full, 0, 512, 0), (n_full + 1, 128, 384, 512)], 896, True)
                    )
                    groups.append(
                        (
                            [(n_full + 2, 256, 256, 0), (n_full + 3, 384, 128, 256)],
                            384,
                            True,
                        )
                    )

                    ot = otps.tile([128, 512], F32, name="ot")
                    lp = lps.tile([128, 512], F32, name="lp", tag="lp")

                    for gi, (chunks, extent, is_diag) in enumerate(groups):
                        st = stps.tile([128, 1024], F32, name="st", tag="stps")
                        for (j, qoff, width, col) in chunks:
                            nc.tensor.matmul(
                                st[:, col : col + width],
                                kt[:, j * 128 : (j + 1) * 128],
                                qt[:, q0 + qoff : q0 + qoff + width],
                                start=True,
                                stop=True,
                            )
                        if is_diag:
                            for (j, qoff, width, col) in chunks:
                                nc.vector.tensor_add(
                                    st[:, col : col + 128],
                                    st[:, col : col + 128],
                                    mask[:],
                                )
                        pt = ptp.tile([128, 1024], F32R, name="pt", tag="pt")
                        nc.scalar.activation(
                            pt[:, 0:extent],
                            st[:, 0:extent],
                            EXPF,
                            bias=zb[:],
                            scale=SCALE,
                        )
                        flush_pv()
                        if gi == 1:
                            flush_evac()

                        def pv(
                            chunks=chunks,
                            ot=ot,
                            lp=lp,
                            pt=pt,
                            n_ch=n_ch,
                            is_last=(gi == len(groups) - 1),
                            b=b,
                            q0=q0,
                        ):
                            for (j, qoff, width, col) in chunks:
                                nc.tensor.matmul(
                                    ot[:, qoff : qoff + width],
                                    vr[:, j * 128 : (j + 1) * 128],
                                    pt[:, col : col + width],
                                    start=(j == 0),
                                    stop=(j == n_ch - 1),
                                )
                                nc.tensor.matmul(
                                    lp[:, qoff : qoff + width],
                                    ones_r[:],
                                    pt[:, col : col + width],
                                    start=(j == 0),
                                    stop=(j == n_ch - 1),
                                )
                            if not is_last:
                                return
                            # ---- evacuation stage 1 (DVE): O^T * (1/l) ----
                            # lp rows are all equal (all-ones stationary)
                            recip = evacp.tile([128, 512], F32, name="recip")
                            nc.vector.reciprocal(recip[:], lp[:])
                            ots = evacp.tile([128, 512], F32R, name="ots")
                            with nc.allow_low_precision(
                                "f32r is full-width fp32 storage"
                            ):
                                nc.vector.tensor_mul(ots[:], ot[:], recip[:])

                            # ---- stage 2 (deferred): transpose + store ----
                            def evac(b=b, q0=q0, ots=ots):
                                trp = lps.tile([128, 512], F32R, name="trp", tag="lp")
                                for s4 in range(4):
                                    nc.tensor.transpose(
                                        trp[:, s4 * 128 : (s4 + 1) * 128],
                                        ots[:, s4 * 128 : (s4 + 1) * 128],
                                        identR[:],
                                    )
                                outsb = evacp.tile([128, 512], F32, name="outsb")
                                nc.vector.tensor_copy(outsb[:], trp[:].bitcast(F32))
                                nc.sync.dma_start(
                                    o_d[b, q0 : q0 + 512, :].rearrange(
                                        "(a p) d -> p a d", p=128
                                    ),
                                    outsb[:].rearrange("p (a d) -> p a d", d=128),
                                )

                            pending_evac[0] = evac

                        pending_pv[0] = pv

            flush_pv()
            flush_evac()
    nc.compile()
    return nc


_NC_CACHE = None


def _get_nc():
    global _NC_CACHE
    if _NC_CACHE is None:
        _NC_CACHE = build()
    return _NC_CACHE


def kernel(query, key, value, _trace=False):
    nc = _get_nc()
    in_maps = []
    for c in range(N_CORES):
        sl = slice(c * BPC, (c + 1) * BPC)
        in_maps.append(
            {
                "q": np.ascontiguousarray(query[sl], dtype=np.float32),
                "k": np.ascontiguousarray(key[sl], dtype=np.float32),
                "v": np.ascontiguousarray(value[sl], dtype=np.float32),
            }
        )
    res = run_bass_kernel_spmd(
        nc, in_maps, core_ids=list(range(N_CORES)), trace=_trace
    )
    out = np.concatenate([res.results[c]["o"] for c in range(N_CORES)], axis=0)
    if _trace:
        return out, res
    return out


# revision 10
# speedup vs baseline: 1.3539x; 1.0688x over previous
"""Causal attention kernel for Trainium2 (Bass/Tile), 8-core SPMD.

Problem: B=16, S=2048, D=128 fp32 causal attention
    scores = Q @ K^T            (per batch)
    scores -= INF * triu(k=1)   (before scaling, as in reference)
    attn = softmax(scores / sqrt(D))
    out = attn @ V

Sharding: batch dim across 8 cores, 2 batches per core, no communication.

Per-core dataflow (per batch, per 512-wide q-block, per 128-wide k-chunk):
    S^T[k, q] = (K^T chunk).T @ Q^T slice      (contract d on partitions)
    diag chunks: += triangular -1e9 mask (DVE)
    P^T = exp(S^T * 1/sqrt(D))                 (ACT, PSUM -> SBUF fp32r)
    O^T[d, q] += V_chunk.T @ P^T chunk          (PSUM accumulate)
    l[q]      += allones.T @ P^T chunk          (rowsum, broadcast on all rows)
    out[q, d] = transpose(O^T * (1/l))          (DVE scale + PE transpose)

All matmuls run in fp32r (full-rate fp32 on the PE; producers round).
Evacuation of q-block N is deferred past q-block N+1's first group so the
in-order PE queue never stalls on the DVE normalize chain.
"""

import os

os.environ.setdefault("MYCRO_LOCAL_CACHE", "1")

import math

import numpy as np

import concourse.bass as bass
import concourse.mybir as mybir
import concourse.tile as tile
from concourse import bacc
from concourse.bass_utils import run_bass_kernel_spmd
from concourse.masks import make_identity

F32 = mybir.dt.float32
F32R = mybir.dt.float32r
EXPF = mybir.ActivationFunctionType.Exp

N_CORES = 8
B = 16
S = 2048
D = 128
BPC = B // N_CORES  # batches per core
SCALE = 1.0 / math.sqrt(float(D))
NEG = -1.0e9
NQB = S // 512  # q blocks per batch
NCH = S // 128  # k chunks per batch


def build():
    nc = bacc.Bacc("TRN2", target_bir_lowering=False, debug=False, num_devices=N_CORES)
    q_d = nc.dram_tensor("q", [BPC, S, D], F32, kind="ExternalInput")
    k_d = nc.dram_tensor("k", [BPC, S, D], F32, kind="ExternalInput")
    v_d = nc.dram_tensor("v", [BPC, S, D], F32, kind="ExternalInput")
    o_d = nc.dram_tensor("o", [BPC, S, D], F32, kind="ExternalOutput")

    with tile.TileContext(nc) as tc:
        with (
            tc.tile_pool(name="const", bufs=1) as constp,
            tc.tile_pool(name="nat", bufs=2) as natp,
            tc.tile_pool(name="tpose", bufs=2) as tposep,
            tc.tile_pool(name="pt", bufs=4) as ptp,
            tc.tile_pool(name="evac", bufs=2) as evacp,
            tc.tile_pool(name="stps", bufs=2, space="PSUM") as stps,
            tc.tile_pool(name="otps", bufs=2, space="PSUM") as otps,
            tc.tile_pool(name="lps", bufs=2, space="PSUM") as lps,
        ):
            # ---- constants ----
            ident32 = constp.tile([128, 128], F32, name="ident32")
            make_identity(nc, ident32[:])
            zb = constp.tile([128, 1], F32, name="zb")
            nc.gpsimd.memset(zb[:], 0.0)
            ones_f = constp.tile([128, 128], F32, name="ones_f")
            nc.gpsimd.memset(ones_f[:], 1.0)
            ones_r = constp.tile([128, 128], F32R, name="ones_r")
            nc.vector.tensor_copy(ones_r[:], ones_f[:])
            identR = constp.tile([128, 128], F32R, name="identR")
            nc.vector.tensor_copy(identR[:], ident32[:])
            # u01[i, k] = 1 iff i < k; with rhs=-1e9*I the product writes the
            # causal mask NEG*[q < k] straight into PSUM on the PE.
            u01_f = constp.tile([128, 128], F32, name="u01_f")
            nc.gpsimd.memset(u01_f[:], 1.0)
            nc.gpsimd.affine_select(
                out=u01_f[:],
                in_=u01_f[:],
                compare_op=mybir.AluOpType.is_ge,
                fill=0.0,
                base=-1,
                pattern=[[1, 128]],
                channel_multiplier=-1,
            )
            u01_r = constp.tile([128, 128], F32R, name="u01_r")
            nc.vector.tensor_copy(u01_r[:], u01_f[:])
            idneg_r = constp.tile([128, 128], F32R, name="idneg_r")
            with nc.allow_low_precision("f32r is full-width fp32 storage"):
                nc.vector.tensor_scalar_mul(idneg_r[:], ident32[:], NEG)

            # HAM warmup: dense PE activity while the first DMAs land
            warm_ps = stps.tile([128, 128], F32, name="warm_ps", tag="stps")
            for _ in range(30):
                nc.tensor.matmul(
                    warm_ps[:], identR[:], identR[:], start=True, stop=True
                )

            # software-pipeline state: PV/rowsum of group g is emitted
            # after S+exp of group g+1 (PE never queues behind exp); the
            # transpose/store tail of q-block N is emitted during N+1.
            pending_pv = [None]
            pending_evac = [None]

            def flush_pv():
                if pending_pv[0] is not None:
                    pending_pv[0]()
                    pending_pv[0] = None

            def flush_evac():
                if pending_evac[0] is not None:
                    pending_evac[0]()
                    pending_evac[0] = None

            for b in range(BPC):
                # ---- load natural-layout tiles ----
                # nat[:, j*128 + d] = X[b, j*128 + p, d]
                q_nat = natp.tile([128, S], F32, name="q_nat")
                k_nat = natp.tile([128, S], F32, name="k_nat")
                v_nat = natp.tile([128, S], F32, name="v_nat")
                nc.sync.dma_start(
                    q_nat[:].rearrange("p (j d) -> p j d", d=128),
                    q_d[b].rearrange("(j p) d -> p j d", p=128),
                )
                nc.sync.dma_start(
                    k_nat[:].rearrange("p (j d) -> p j d", d=128),
                    k_d[b].rearrange("(j p) d -> p j d", p=128),
                )
                nc.sync.dma_start(
                    v_nat[:].rearrange("p (j d) -> p j d", d=128),
                    v_d[b].rearrange("(j p) d -> p j d", p=128),
                )

                # ---- build Q^T, K^T (fp32r) via PE transposes ----
                qt = tposep.tile([128, S], F32R, name="qt")
                kt = tposep.tile([128, S], F32R, name="kt")
                tgroup_i = 0
                for (nat, tt) in ((q_nat, qt), (k_nat, kt)):
                    for j4 in range(NCH // 4):
                        tp = stps.tile([128, 512], F32, name="tp", tag="stps")
                        for m in range(4):
                            j = j4 * 4 + m
                            nc.tensor.transpose(
                                tp[:, m * 128 : (m + 1) * 128],
                                nat[:, j * 128 : (j + 1) * 128],
                                ident32[:],
                            )
                        nc.vector.tensor_copy(
                            tt[:, j4 * 512 : (j4 + 1) * 512], tp[:]
                        )
                        # previous batch's tail overlaps the transpose work
                        if tgroup_i == 0:
                            flush_pv()
                        elif tgroup_i == 1:
                            flush_evac()
                        tgroup_i += 1

                # ---- round V to fp32r ----
                vr = tposep.tile([128, S], F32R, name="vr")
                nc.vector.tensor_copy(vr[:], v_nat[:])

                # ---- q blocks ----
                for qb in range(NQB):
                    n_full = 4 * qb
                    n_ch = n_full + 4
                    q0 = qb * 512

                    # (chunks, extent, is_diag); st tiles are [128, 1024]
                    groups = []
                    jf = 0
                    while jf < n_full:
                        g = min(2, n_full - jf)
                        groups.append(
                            (
                                [(jf + c, 0, 512, c * 512) for c in range(g)],
                                g * 512,
                                False,
                            )
                        )
                        jf += g
                    groups.append(
                        ([(n_full, 0, 512, 0), (n_full + 1, 128, 384, 512)], 896, True)
                    )
                    groups.append(
                        (
                            [(n_full + 2, 256, 256, 0), (n_full + 3, 384, 128, 256)],
                            384,
                            True,
                        )
                    )

                    ot = otps.tile([128, 512], F32, name="ot")
                    lp = lps.tile([128, 512], F32, name="lp", tag="lp")

                    for gi, (chunks, extent, is_diag) in enumerate(groups):
                        st = stps.tile([128, 1024], F32, name="st", tag="stps")
                        for (j, qoff, width, col) in chunks:
                            if is_diag:
                                # write NEG*[q<k] into the first 128 cols,
                                # then accumulate the scores on top
                                nc.tensor.matmul(
                                    st[:, col : col + 128],
                                    u01_r[:],
                                    idneg_r[:],
                                    start=True,
                                    stop=False,
                                )
                            nc.tensor.matmul(
                                st[:, col : col + width],
                                kt[:, j * 128 : (j + 1) * 128],
                                qt[:, q0 + qoff : q0 + qoff + width],
                                start=not is_diag,
                                stop=True,
                            )
                        pt = ptp.tile([128, 1024], F32R, name="pt", tag="pt")
                        nc.scalar.activation(
                            pt[:, 0:extent],
                            st[:, 0:extent],
                            EXPF,
                            bias=zb[:],
                            scale=SCALE,
                        )
                        flush_pv()
                        if gi == 1:
                            flush_evac()

                        def pv(
                            chunks=chunks,
                            ot=ot,
                            lp=lp,
                            pt=pt,
                            n_ch=n_ch,
                            is_last=(gi == len(groups) - 1),
                            b=b,
                            q0=q0,
                        ):
                            for (j, qoff, width, col) in chunks:
                                nc.tensor.matmul(
                                    ot[:, qoff : qoff + width],
                                    vr[:, j * 128 : (j + 1) * 128],
                                    pt[:, col : col + width],
                                    start=(j == 0),
                                    stop=(j == n_ch - 1),
                                )
                                nc.tensor.matmul(
                                    lp[:, qoff : qoff + width],
                                    ones_r[:],
                                    pt[:, col : col + width],
                                    start=(j == 0),
                                    stop=(j == n_ch - 1),
                                )
                            if not is_last:
                                return
                            # ---- evacuation stage 1 (DVE): O^T * (1/l) ----
                            # lp rows are all equal (all-ones stationary)
                            recip = evacp.tile([128, 512], F32, name="recip")
                            nc.vector.reciprocal(recip[:], lp[:])
                            ots = evacp.tile([128, 512], F32R, name="ots")
                            with nc.allow_low_precision(
                                "f32r is full-width fp32 storage"
                            ):
                                nc.vector.tensor_mul(ots[:], ot[:], recip[:])

                            # ---- stage 2 (deferred): transpose + store ----
                            def evac(b=b, q0=q0, ots=ots):
                                trp = lps.tile([128, 512], F32R, name="trp", tag="lp")
                                for s4 in range(4):
                                    nc.tensor.transpose(
                                        trp[:, s4 * 128 : (s4 + 1) * 128],
                                        ots[:, s4 * 128 : (s4 + 1) * 128],
                                        identR[:],
                                    )
                                outsb = evacp.tile([128, 512], F32, name="outsb")
                                nc.vector.tensor_copy(outsb[:], trp[:].bitcast(F32))
                                nc.sync.dma_start(
                                    o_d[b, q0 : q0 + 512, :].rearrange(
                                        "(a p) d -> p a d", p=128
                                    ),
                                    outsb[:].rearrange("p (a d) -> p a d", d=128),
                                )

                            pending_evac[0] = evac

                        pending_pv[0] = pv

            flush_pv()
            flush_evac()
    nc.compile()
    return nc


_NC_CACHE = None


def _get_nc():
    global _NC_CACHE
    if _NC_CACHE is None:
        _NC_CACHE = build()
    return _NC_CACHE


def kernel(query, key, value, _trace=False):
    nc = _get_nc()
    in_maps = []
    for c in range(N_CORES):
        sl = slice(c * BPC, (c + 1) * BPC)
        in_maps.append(
            {
                "q": np.ascontiguousarray(query[sl], dtype=np.float32),
                "k": np.ascontiguousarray(key[sl], dtype=np.float32),
                "v": np.ascontiguousarray(value[sl], dtype=np.float32),
            }
        )
    res = run_bass_kernel_spmd(
        nc, in_maps, core_ids=list(range(N_CORES)), trace=_trace
    )
    out = np.concatenate([res.results[c]["o"] for c in range(N_CORES)], axis=0)
    if _trace:
        return out, res
    return out
